# revision 14
# baseline (speedup 1.0000x reference)
"""Trainium2 Bass kernel for nn_MessageDecoder (LSTM greedy decoder).

Data parallel over batch (8 cores, 4096 rows each). Per core:
  - h state transposed in SBUF: h_T [H=512 (4x128 part-tiles), BC].
  - c state streamed through a DRAM scratch tensor (SBUF cannot hold both).
  - Per step: gates_T(psum) = W_hhT-stationary matmuls over h_T + one-hot matmul
    against T0 = emb @ W_ih^T + b_ih + b_hh (precomputed on host, 67 x 2048);
    the one-hot of the previous action is built on-chip.
  - logits batch-major (lhsT = h_T slices); argmax via DVE max/max_index;
    per-step dumps of (action, max, sum exp, sum exp*l); lp/entropy on host.
"""
import sys
sys.path.insert(0, '/opt/trn_rl_repo')
import numpy as np

import concourse.bass as bass  # noqa: F401
import concourse.mybir as mybir
from concourse import bacc
from concourse.tile import TileContext
from concourse.bass_utils import run_bass_kernel_spmd
from concourse.alu_op_type import AluOpType

F32 = mybir.dt.float32
F32R = mybir.dt.float32r
I32 = mybir.dt.int32
U32 = mybir.dt.uint32
AF = mybir.ActivationFunctionType

H = 512
NACT = 65
V = 67
PAD = 65
BOS = 66
N_CORES = 8


def build_nc(BC=4096, L=20, mm_mode="fp32"):
    NT = BC // 128
    NCH = BC // 512
    assert BC % 512 == 0

    nc = bacc.Bacc("TRN2", target_bir_lowering=False, debug=False)

    d_enc = nc.dram_tensor("enc_t", (128, 4, BC), F32, kind="ExternalInput")
    d_whh = nc.dram_tensor("whh", (128, 4, 2048), F32, kind="ExternalInput")
    d_t0 = nc.dram_tensor("t0", (V, 2048), F32, kind="ExternalInput")
    d_wa = nc.dram_tensor("wa", (128, 4, NACT), F32, kind="ExternalInput")
    d_wc = nc.dram_tensor("wc", (128, 4, H), F32, kind="ExternalInput")
    d_wh = nc.dram_tensor("wh", (128, 4, H), F32, kind="ExternalInput")
    d_bc = nc.dram_tensor("bc4", (128, 4), F32, kind="ExternalInput")
    d_bh = nc.dram_tensor("bh4", (128, 4), F32, kind="ExternalInput")
    d_ba = nc.dram_tensor("ba_r", (1, NACT), F32, kind="ExternalInput")
    d_iota = nc.dram_tensor("iota_c", (V, 1), F32, kind="ExternalInput")
    d_ones128 = nc.dram_tensor("ones128", (1, 128), F32, kind="ExternalInput")
    d_sel = nc.dram_tensor("sel", (128, 4, 128), F32, kind="ExternalInput")
    d_onesbc = nc.dram_tensor("ones_bc", (128, V), F32, kind="ExternalInput")
    d_padc = nc.dram_tensor("padc", (128, NT), F32, kind="ExternalInput")
    d_oh0 = nc.dram_tensor("oh0", (V, BC), F32, kind="ExternalInput")

    o_msg = nc.dram_tensor("msg_o", (128, L * NT), I32, kind="ExternalOutput")
    o_s = nc.dram_tensor("s_o", (128, L * NT), F32, kind="ExternalOutput")
    o_t = nc.dram_tensor("t_o", (128, L * NT), F32, kind="ExternalOutput")
    o_mx = nc.dram_tensor("mx_o", (128, L * NT), F32, kind="ExternalOutput")

    from contextlib import ExitStack
    with TileContext(nc) as tc, ExitStack() as ctx:
        const = ctx.enter_context(tc.tile_pool(name="const", bufs=1))
        state = ctx.enter_context(tc.tile_pool(name="state", bufs=1))
        dram = ctx.enter_context(tc.tile_pool(name="dram", bufs=1, space="DRAM"))

        hT = state.tile([128, 4, BC], F32, tag="hT")
        oh = state.tile([V, BC], F32, tag="oh")
        stopped = state.tile([128, NT], I32, tag="stp")
        c_dr = dram.tile([128, 4, BC], F32, tag="c_dr")

        whh_sb = const.tile([128, 4, 2048], F32, tag="whh")
        nc.sync.dma_start(whh_sb[:], d_whh.ap())
        t0_sb = const.tile([V, 2048], F32, tag="t0")
        nc.sync.dma_start(t0_sb[:], d_t0.ap())
        wa_sb = const.tile([128, 4, NACT], F32, tag="wa")
        nc.sync.dma_start(wa_sb[:], d_wa.ap())
        ba_sb = const.tile([1, NACT], F32, tag="ba")
        nc.sync.dma_start(ba_sb[:], d_ba.ap())
        iota_sb = const.tile([V, 1], F32, tag="iota")
        nc.sync.dma_start(iota_sb[:], d_iota.ap())
        ones128_sb = const.tile([1, 128], F32, tag="o128")
        nc.sync.dma_start(ones128_sb[:], d_ones128.ap())
        sel_sb = const.tile([128, 4, 128], F32, tag="sel")
        nc.sync.dma_start(sel_sb[:], d_sel.ap())
        ones_bc_sb = const.tile([128, V], F32, tag="obc")
        nc.sync.dma_start(ones_bc_sb[:], d_onesbc.ap())
        padc_sb = const.tile([128, NT], F32, tag="padc")
        nc.sync.dma_start(padc_sb[:], d_padc.ap())
        bc_sb = const.tile([128, 4], F32, tag="bc")
        nc.sync.dma_start(bc_sb[:], d_bc.ap())
        bh_sb = const.tile([128, 4], F32, tag="bh")
        nc.sync.dma_start(bh_sb[:], d_bh.ap())

        # ---------------- init ----------------
        with tc.tile_pool(name="initp", bufs=2) as initp, \
             tc.tile_pool(name="ipsum", bufs=4, space="PSUM") as ipp:
            wc_sb = initp.tile([128, 4, H], F32, tag="wc", bufs=1)
            nc.sync.dma_start(wc_sb[:], d_wc.ap())
            wh_sb = initp.tile([128, 4, H], F32, tag="wh", bufs=1)
            nc.sync.dma_start(wh_sb[:], d_wh.ap())
            for nb in range(NCH):
                nsl = slice(nb * 512, (nb + 1) * 512)
                enc = initp.tile([128, 4, 512], F32, tag="enc")
                nc.sync.dma_start(enc[:], d_enc.ap()[:, :, nsl])
                for hk in range(4):
                    ph = ipp.tile([128, 512], F32, tag="iph")
                    pc = ipp.tile([128, 512], F32, tag="ipc")
                    for k in range(4):
                        nc.tensor.matmul(ph[:], wc_sb[:, k, hk * 128:(hk + 1) * 128],
                                         enc[:, k, :], start=(k == 0), stop=(k == 3))
                    for k in range(4):
                        nc.tensor.matmul(pc[:], wh_sb[:, k, hk * 128:(hk + 1) * 128],
                                         enc[:, k, :], start=(k == 0), stop=(k == 3))
                    nc.scalar.activation(hT[:, hk, nsl], ph[:], AF.Identity,
                                         bias=bc_sb[:, hk:hk + 1])
                    c0 = initp.tile([128, 512], F32, tag="c0")
                    nc.scalar.activation(c0[:], pc[:], AF.Identity,
                                         bias=bh_sb[:, hk:hk + 1])
                    nc.sync.dma_start(c_dr[:, hk, nsl], c0[:])
        nc.sync.dma_start(oh[:], d_oh0.ap())
        nc.vector.memset(stopped[:], 0)

        # ---------------- main decode loop ----------------
        gpool = ctx.enter_context(tc.tile_pool(name="gp", bufs=2))
        spool = ctx.enter_context(tc.tile_pool(name="sp", bufs=4))
        cpool = ctx.enter_context(tc.tile_pool(name="cp", bufs=3))
        curp = ctx.enter_context(tc.tile_pool(name="cur", bufs=2))
        ppg = ctx.enter_context(tc.tile_pool(name="ppg", bufs=5, space="PSUM"))
        ppl = ctx.enter_context(tc.tile_pool(name="ppl", bufs=1, space="PSUM"))
        ppm = ctx.enter_context(tc.tile_pool(name="ppm", bufs=2, space="PSUM"))

        for s in range(L):
            msg_c = curp.tile([128, NT], F32, tag="msgc")
            s_c = curp.tile([128, NT], F32, tag="sc")
            t_c = curp.tile([128, NT], F32, tag="tc_")
            mx_c = curp.tile([128, NT], F32, tag="mxc")
            osl = slice(s * NT, (s + 1) * NT)

            for nb in range(NCH):
                nsl = slice(nb * 512, (nb + 1) * 512)
                hw_pend = []
                for hk in range(4):
                    psums = []
                    for g in range(4):
                        p = ppg.tile([128, 512], F32, tag="g")
                        m = g * 4 + hk
                        for k in range(4):
                            nc.tensor.matmul(p[:], whh_sb[:, k, m * 128:(m + 1) * 128],
                                             hT[:, k, nsl], start=(k == 0), stop=False)
                        nc.tensor.matmul(p[:], t0_sb[:, m * 128:(m + 1) * 128],
                                         oh[:, nsl], start=False, stop=True)
                        psums.append(p)
                    # LSTM cell; h-writes deferred until all gate matmuls of
                    # this chunk have consumed the old h.
                    c_in = cpool.tile([128, 512], F32, tag="ci")
                    nc.sync.dma_start(c_in[:], c_dr[:, hk, nsl])
                    si = gpool.tile([128, 512], F32, tag="si")
                    nc.scalar.activation(si[:], psums[0][:], AF.Sigmoid)
                    sf = gpool.tile([128, 512], F32, tag="sf")
                    nc.scalar.activation(sf[:], psums[1][:], AF.Sigmoid)
                    tg = gpool.tile([128, 512], F32, tag="tg")
                    nc.scalar.activation(tg[:], psums[2][:], AF.Tanh)
                    so = gpool.tile([128, 512], F32, tag="so", bufs=6)
                    nc.scalar.activation(so[:], psums[3][:], AF.Sigmoid)
                    nc.vector.tensor_mul(si[:], si[:], tg[:])
                    nc.vector.tensor_mul(sf[:], sf[:], c_in[:])
                    c_out = cpool.tile([128, 512], F32, tag="co")
                    nc.vector.tensor_add(c_out[:], si[:], sf[:])
                    nc.sync.dma_start(c_dr[:, hk, nsl], c_out[:])
                    tc_ = gpool.tile([128, 512], F32, tag="tc", bufs=6)
                    nc.scalar.activation(tc_[:], c_out[:], AF.Tanh)
                    hw_pend.append((hk, so, tc_))
                for hk, so, tc_ in hw_pend:
                    nc.vector.tensor_mul(hT[:, hk, nsl], so[:], tc_[:])

                # logits + argmax for the 4 batch tiles of this chunk
                for ti in range(4):
                    t = nb * 4 + ti
                    tsl = slice(t * 128, (t + 1) * 128)
                    pl = ppl.tile([128, NACT], F32, tag="l")
                    for k in range(4):
                        nc.tensor.matmul(pl[:], hT[:, k, tsl], wa_sb[:, k, :],
                                         start=(k == 0), stop=False)
                    nc.tensor.matmul(pl[:], ones128_sb[:], ba_sb[:],
                                     start=False, stop=True)
                    lsb = spool.tile([128, NACT], F32, tag="lsb")
                    nc.vector.tensor_copy(lsb[:], pl[:])
                    mx8 = spool.tile([128, 8], F32, tag="mx8")
                    nc.vector.max(mx8[:], lsb[:])
                    mi8 = spool.tile([128, 8], U32, tag="mi8")
                    nc.vector.max_index(mi8[:], mx8[:], lsb[:])
                    nc.vector.tensor_copy(mx_c[:, t:t + 1], mx8[:, 0:1])
                    ex = spool.tile([128, NACT], F32, tag="ex")
                    nc.scalar.activation(ex[:], lsb[:], AF.Exp,
                                         accum_out=s_c[:, t:t + 1])
                    dm = spool.tile([128, NACT], F32, tag="dm")
                    nc.vector.tensor_mul(dm[:], ex[:], lsb[:])
                    nc.vector.tensor_reduce(out=t_c[:, t:t + 1], in_=dm[:],
                                            axis=mybir.AxisListType.X,
                                            op=AluOpType.add)
                    idxf = spool.tile([128, 1], F32, tag="ix")
                    nc.vector.tensor_copy(idxf[:], mi8[:, 0:1])
                    nc.vector.select(msg_c[:, t:t + 1], stopped[:, t:t + 1],
                                     padc_sb[:, t:t + 1], idxf[:])

                # next one-hot for this chunk:
                # rhsb[p, tau, q] = sel[p, tau, q] * action[p, tau]
                # pb[v, n] = sum_p rhsb[p, n] = action[p(n), tau(n)]
                if s < L - 1:
                    rhsb = spool.tile([128, 4, 128], F32, tag="rb", bufs=2)
                    asl = msg_c[:, nb * 4:nb * 4 + 4]
                    nc.vector.tensor_mul(
                        rhsb[:], sel_sb[:],
                        asl[:, :, None].broadcast_to([128, 4, 128]))
                    pb = ppm.tile([V, 512], F32, tag="bc")
                    nc.tensor.matmul(pb[:], ones_bc_sb[:],
                                     rhsb[:].rearrange("p a b -> p (a b)"),
                                     start=True, stop=True)
                    nc.vector.tensor_scalar(
                        out=oh[:, nsl], in0=pb[:], scalar1=iota_sb[:],
                        scalar2=None, op0=AluOpType.is_equal)

            # stopped |= (action == EOS); dump per-step outputs
            eos = curp.tile([128, NT], I32, tag="eo")
            nc.vector.tensor_scalar(out=eos[:], in0=msg_c[:], scalar1=0.0,
                                    scalar2=None, op0=AluOpType.is_equal)
            nc.vector.tensor_max(stopped[:], stopped[:], eos[:])
            msg_i = curp.tile([128, NT], I32, tag="msgi")
            nc.vector.tensor_copy(msg_i[:], msg_c[:])
            nc.sync.dma_start(o_msg.ap()[:, osl], msg_i[:])
            nc.sync.dma_start(o_s.ap()[:, osl], s_c[:])
            nc.sync.dma_start(o_t.ap()[:, osl], t_c[:])
            nc.sync.dma_start(o_mx.ap()[:, osl], mx_c[:])

    nc.compile()
    return nc


def prep_inputs(encoded, emb, W_ih, W_hh, b_ih, b_hh, Wc, bc, Wh, bh, Wa, ba,
                BC=4096, n_cores=8):
    f = np.float32
    NT = BC // 128
    T0 = (emb.astype(np.float64) @ W_ih.T.astype(np.float64)
          + b_ih.astype(np.float64) + b_hh.astype(np.float64)).astype(f)
    whh = np.ascontiguousarray(W_hh.T).reshape(4, 128, 2048).transpose(1, 0, 2)
    wa = np.ascontiguousarray(Wa.T).reshape(4, 128, NACT).transpose(1, 0, 2)
    wc = np.ascontiguousarray(Wc.T).reshape(4, 128, H).transpose(1, 0, 2)
    wh = np.ascontiguousarray(Wh.T).reshape(4, 128, H).transpose(1, 0, 2)
    oh0 = np.zeros((V, BC), f)
    oh0[BOS, :] = 1.0
    common = {
        "whh": np.ascontiguousarray(whh, f),
        "t0": np.ascontiguousarray(T0, f),
        "wa": np.ascontiguousarray(wa, f),
        "wc": np.ascontiguousarray(wc, f),
        "wh": np.ascontiguousarray(wh, f),
        "bc4": np.ascontiguousarray(bc.reshape(4, 128).T, f),
        "bh4": np.ascontiguousarray(bh.reshape(4, 128).T, f),
        "ba_r": np.ascontiguousarray(ba.reshape(1, NACT), f),
        "iota_c": np.arange(V, dtype=f).reshape(V, 1),
        "ones128": np.ones((1, 128), f),
        "sel": np.ascontiguousarray(
            np.broadcast_to(np.eye(128, dtype=f)[:, None, :], (128, 4, 128))),
        "ones_bc": np.ones((128, V), f),
        "padc": np.full((128, NT), float(PAD), f),
        "oh0": oh0,
    }
    encT = np.ascontiguousarray(encoded.T)
    in_maps = []
    for c in range(n_cores):
        sl = encT[:, c * BC:(c + 1) * BC]
        enc_t = np.ascontiguousarray(sl.reshape(4, 128, BC).transpose(1, 0, 2), f)
        in_maps.append({**common, "enc_t": enc_t})
    return in_maps


def unpack_outputs(outs, BC=4096, L=20):
    """Per-core packed dumps -> (entropy, log_probs, message, message_len)."""
    NT = BC // 128
    ents, lps, msgs, lens = [], [], [], []
    for o in outs:
        def unpk(a):
            return a.reshape(128, L, NT).transpose(2, 0, 1).reshape(BC, L)
        msg = unpk(o["msg_o"])
        S = unpk(o["s_o"]).astype(np.float64)
        T = unpk(o["t_o"]).astype(np.float64)
        mx = unpk(o["mx_o"]).astype(np.float64)
        alive = (msg != PAD)
        lnS = np.log(S)
        lp = ((mx - lnS) * alive).astype(np.float32)
        ent_steps = (lnS - T / S) * alive
        mlen = alive.sum(1, keepdims=True).astype(np.int32)
        entropy = (ent_steps.sum(1, keepdims=True) / mlen).astype(np.float32)
        msgs.append(msg.astype(np.int32)); lps.append(lp)
        ents.append(entropy); lens.append(mlen)
    return (np.concatenate(ents), np.concatenate(lps),
            np.concatenate(msgs), np.concatenate(lens))


def kernel(encoded, emb, W_ih, W_hh, b_ih, b_hh, Wc, bc, Wh, bh, Wa, ba):
    B = encoded.shape[0]
    BC = B // N_CORES
    L = 20
    nc = build_nc(BC=BC, L=L)
    in_maps = prep_inputs(encoded, emb, W_ih, W_hh, b_ih, b_hh,
                          Wc, bc, Wh, bh, Wa, ba, BC=BC, n_cores=N_CORES)
    res = run_bass_kernel_spmd(nc, in_maps, core_ids=list(range(N_CORES)))
    return unpack_outputs(res.results, BC=BC, L=L)
